# revision 12
# baseline (speedup 1.0000x reference)
"""Trainium2 Bass kernel for nn_CustomLoss_30743375905383.

loss = sum_i[ (p0-(1-t))^2 + (p1-t)^2 + 2*[wrong] ] / N
  where wrong = (t==0 ? p0<p1 : p1<p0)

Math (s' = 2t-1 in {-1,+1}, e = p0-p1, q = s'*e):
  sq_i  = (1-p0)^2 + p1^2 + 2*t*e    and   2t*e = e + q
  wrong = q > 0 ; quantization ties (q==0) are counted via alternating
          is_gt / is_ge per tile (a tie's true penalty is 0 or 2 with
          equal odds, so alternating strict/non-strict is unbiased).
  loss_sum = SA + SB + E + Q + G2
    SA = sum (1-p0)^2   [ScalarE Square scale=-1 bias=1, native accum]
    SB = sum p1^2       [ScalarE Square, native accum]
    E  = sum e          [TensorE ones-matmul into PSUM]
    Q  = sum q          [TensorE ones-matmul into PSUM]
    G2 = sum 2*[q>0]    [VectorE tensor_scalar+accum, 1x fused]

Data-parallel over 8 NeuronCores: core c handles N/8 consecutive rows.

Engine notes (measured this session):
  - DVE ops with accum_out run 1x; plain bf16 TT runs 2x. So e and q
    are plain 2x ops and their sums go to the otherwise-idle TensorE
    (ones-stationary matmuls, 512-col chunks = 1 PSUM bank each,
    back-to-back so the PE clock ramps from 1.2 to 2.4 GHz).
  - The penalty op must see q elementwise anyway, so it uses the fused
    tensor_scalar+accum (1x) - same cost as a separate sum pass but no
    extra g tensor or matmuls.
  - ScalarE activations accumulate for free; it takes both squares.

Bandwidth: pred planes bf16 in HBM (8 MiB/core, HWDGE, half-plane
loads = 16 KiB/row descriptors); s' fp8 in HBM (2 MiB) upcast to bf16
in-flight by SWDGE so the q op stays in DVE 2x mode.
"""

import sys

if "/opt/trn_rl_repo" not in sys.path:
    sys.path.insert(0, "/opt/trn_rl_repo")

import numpy as np
import ml_dtypes
import concourse.bass as bass
import concourse.mybir as mybir
import concourse.tile as tile
from concourse.bass_utils import run_bass_kernel_spmd

F32 = mybir.dt.float32
BF16 = mybir.dt.bfloat16
F8 = mybir.dt.float8e4
AF = mybir.ActivationFunctionType
ALU = mybir.AluOpType

P = 128                          # SBUF partitions
N_TOTAL = 16777216
N_CORES = 8
R = N_TOTAL // N_CORES           # pairs per core = 2097152
W = R // P                       # pairs per partition = 16384

TILE_C = 4096                    # pair-columns per compute tile
NT = W // TILE_C                 # 4 compute tiles
HALF = W // 2                    # 8192 cols per DMA buffer (16 KiB rows)
MM_C = 512                       # matmul chunk = one PSUM bank row

IO_BUFS = 2
MID_BUFS = 2


def _split_excess_waits(nc, max_waits=1):
    """This walrus build's CoreV3 codegen caps sem-wait commands per
    instruction; split excess waits onto preceding same-engine no-ops.
    Engines run their stream in order and the waits are monotonic
    sem-ge conditions, so sequential chunked waits are equivalent."""
    counter = [0]

    def fresh_name(base):
        counter[0] += 1
        return f"{base}-wsplit{counter[0]}"

    for fn in nc.m.functions:
        for bb in fn.blocks:
            out = []
            changed = False
            for inst in bb.instructions:
                si = inst.sync_info
                waits = list(si.on_wait) if si is not None else []
                if len(waits) > max_waits:
                    changed = True
                    head, tail = waits[:-max_waits], waits[-max_waits:]
                    for i in range(0, len(head), max_waits):
                        out.append(mybir.InstNoOp(
                            name=fresh_name(inst.name),
                            sync_info=mybir.SyncInfo(
                                on_wait=head[i:i + max_waits], on_update=[]),
                            bass_nofuse=True,
                            engine=inst.engine,
                        ))
                    inst.sync_info = mybir.SyncInfo(
                        on_wait=tail, on_update=list(si.on_update))
                out.append(inst)
            if changed:
                bb.instructions = out


def _build(io_bufs=IO_BUFS, mid_bufs=MID_BUFS):
    C = TILE_C
    nt = NT
    nc = bass.Bass(trn_type="TRN2", target_bir_lowering=False, debug=False)
    p0 = nc.dram_tensor("p0", [P, W], BF16, kind="ExternalInput").ap()
    p1 = nc.dram_tensor("p1", [P, W], BF16, kind="ExternalInput").ap()
    sg = nc.dram_tensor("sg", [P, W], F8, kind="ExternalInput").ap()
    # [ SA | SB | G2 ] per tile from the ScalarE / VectorE accumulators
    out_acc = nc.dram_tensor("out_acc", [P, 3 * nt], F32,
                             kind="ExternalOutput").ap()
    # [ E | Q ] PE psum rows (partition 0)
    out_red = nc.dram_tensor("out_red", [1, 2 * MM_C], F32,
                             kind="ExternalOutput").ap()

    ones = nc.const_aps.tensor(1.0, (P, 1), BF16)

    with tile.TileContext(nc) as tc:
        with tc.tile_pool(name="io", bufs=io_bufs) as io_pool, \
             tc.tile_pool(name="mid", bufs=mid_bufs) as mid_pool, \
             tc.tile_pool(name="accs", bufs=1) as acc_pool, \
             tc.psum_pool(name="red", bufs=1) as red_pool:
            acc = acc_pool.tile([P, 3 * nt], F32)
            red = red_pool.tile([1, 2 * MM_C], F32)
            red_sb = acc_pool.tile([1, 2 * MM_C], F32)
            for i in range(nt):
                h, off = divmod(i, 2)
                off *= C
                if off == 0:
                    # one DMA per tensor per half-plane: 16 KiB per
                    # partition row keeps the SDMA engines at line rate
                    P0b = io_pool.tile([P, HALF], BF16, tag="P0b")
                    P1b = io_pool.tile([P, HALF], BF16, tag="P1b")
                    Sb = io_pool.tile([P, HALF], BF16, tag="Sb")
                    hs = slice(h * HALF, (h + 1) * HALF)
                    nc.sync.dma_start(P0b[:], p0[:, hs])
                    nc.sync.dma_start(P1b[:], p1[:, hs])
                    # SWDGE cast-DMA: fp8 in HBM -> bf16 in SBUF
                    nc.gpsimd.dma_start(Sb[:], sg[:, hs])
                cs = slice(off, off + C)

                scrA = mid_pool.tile([P, C], BF16, tag="scrA")
                scrG = mid_pool.tile([P, C], BF16, tag="scrG")
                eT = mid_pool.tile([P, C], BF16, tag="eT")
                qT = mid_pool.tile([P, C], BF16, tag="qT")

                # SA += sum (1 - p0)^2           [ScalarE, native accum]
                nc.scalar.activation(scrA[:], P0b[:, cs], AF.Square,
                                     bias=1.0, scale=-1.0,
                                     accum_out=acc[:, i:i + 1])
                # SB += sum p1^2                 [ScalarE, native accum]
                nc.scalar.activation(scrA[:], P1b[:, cs], AF.Square,
                                     accum_out=acc[:, nt + i:nt + i + 1])
                # e = p0 - p1                    [VectorE TT 2x]
                nc.vector.tensor_tensor(eT[:], P0b[:, cs], P1b[:, cs],
                                        ALU.subtract)
                # q = s' * e                     [VectorE TT 2x]
                nc.vector.tensor_tensor(qT[:], Sb[:, cs], eT[:], ALU.mult)
                # G += sum [q > 0]  (alt >= on odd tiles: unbiased ties;
                # op1 is the reduce op of the fused accumulate; the 2x
                # penalty scale is applied on the host)
                cmp = ALU.is_gt if i % 2 == 0 else ALU.is_ge
                nc.vector.tensor_scalar(
                    scrG[:], qT[:], 0.0, None, cmp, ALU.add,
                    accum_out=acc[:, 2 * nt + i:2 * nt + i + 1])

                # E, Q: ones^T @ 512-col chunks accumulated in PSUM.
                # e-chunks then q-chunks back-to-back (16 matmuls) so
                # the PE stays busy and ramps to its 2.4 GHz pstate.
                for ridx, src in ((0, eT), (1, qT)):
                    for mm in range(C // MM_C):
                        nc.tensor.matmul(
                            red[:, ridx * MM_C:(ridx + 1) * MM_C],
                            ones,
                            src[:, mm * MM_C:(mm + 1) * MM_C],
                            start=(i == 0 and mm == 0),
                            stop=(i == nt - 1 and mm == C // MM_C - 1),
                        )

            # PSUM has no DMA route: bounce through SBUF.
            nc.vector.tensor_copy(red_sb[:], red[:])
            nc.sync.dma_start(out_acc[:], acc[:])
            nc.sync.dma_start(out_red[:], red_sb[:])

    _split_excess_waits(nc, max_waits=1)
    return nc, nt


_CACHE = {}


def _get_program():
    if "prog" not in _CACHE:
        _CACHE["prog"] = _build()
    return _CACHE["prog"]


def kernel(pred, target):
    pred = np.asarray(pred)
    target = np.asarray(target)
    assert pred.shape == (N_TOTAL, 2) and pred.dtype == np.float32
    bf16 = ml_dtypes.bfloat16
    f8 = ml_dtypes.float8_e4m3
    p0_h = np.ascontiguousarray(pred[:, 0]).astype(bf16)
    p1_h = np.ascontiguousarray(pred[:, 1]).astype(bf16)
    s_h = (2 * target.astype(np.int32) - 1).astype(np.float32).astype(f8)

    nc, nt = _get_program()
    in_maps = []
    for c in range(N_CORES):
        sl = slice(c * R, (c + 1) * R)
        in_maps.append({
            "p0": p0_h[sl].reshape(P, W),
            "p1": p1_h[sl].reshape(P, W),
            "sg": s_h[sl].reshape(P, W),
        })

    res = run_bass_kernel_spmd(nc, in_maps, list(range(N_CORES)))

    total = 0.0
    for r in res.results:
        acc = np.asarray(r["out_acc"]).astype(np.float64)
        red = np.asarray(r["out_red"]).astype(np.float64)
        total += (acc[:, :2 * nt].sum()          # SA + SB
                  + 2.0 * acc[:, 2 * nt:].sum()  # penalty 2*G
                  + red.sum())                   # E + Q
    return np.float32(total / N_TOTAL)


# revision 13
# speedup vs baseline: 1.1222x; 1.1222x over previous
"""Trainium2 Bass kernel for nn_CustomLoss_30743375905383.

loss = sum_i[ (p0-(1-t))^2 + (p1-t)^2 + 2*[wrong] ] / N
  where wrong = (t==0 ? p0<p1 : p1<p0)

Math (sigma = 2t in {0,2}, e = p0-p1, u = sigma*e):
  sq_i  = (1-p0)^2 + p1^2 + 2*t*e   and   2*t*e = u
  wrong = u > e   (t=0: 0>e means e<0; t=1: 2e>e means e>0)
  ties (u==e, only possible after quantization) are counted with
  alternating is_gt / is_ge per tile: a tie's true penalty is 0 or 2
  with equal odds, so alternating strict/non-strict is unbiased.
  loss_sum = SA + SB + sum(u) + 2*sum(b),  b = [wrong]
    SA = sum (1-p0)^2   [ScalarE Square scale=-1 bias=1, native accum]
    SB = sum p1^2       [ScalarE Square, native accum]
    RED = sum(u) + 2*sum(b)  [TensorE: ones-stationary matmuls for
          u-chunks, twos-stationary for b-chunks, one PSUM row]

Data-parallel over 8 NeuronCores: core c handles N/8 consecutive rows.

Engine notes (measured this session):
  - DVE accum_out forces 1x; plain bf16 TT runs 2x. So VectorE does
    only three plain 2x ops per tile (e, u, b) and every reduction
    lives elsewhere: squares on ScalarE's free activation accumulator,
    u and b on the otherwise-idle TensorE (512-col chunks = 1 PSUM
    bank, back-to-back so the PE clock ramps 1.2 -> 2.4 GHz).
  - Folding the penalty scale into a twos-stationary avoids a fourth
    DVE op and a second host-side correction.

Bandwidth: pred planes bf16 in HBM (8 MiB/core, HWDGE); sigma fp8 in
HBM (2 MiB) upcast to bf16 in-flight by SWDGE so u stays in DVE 2x
mode. Per-tile DMAs (4096 cols) keep the pipeline ramp short.
"""

import sys

if "/opt/trn_rl_repo" not in sys.path:
    sys.path.insert(0, "/opt/trn_rl_repo")

import numpy as np
import ml_dtypes
import concourse.bass as bass
import concourse.mybir as mybir
import concourse.tile as tile
from concourse.bass_utils import run_bass_kernel_spmd

F32 = mybir.dt.float32
BF16 = mybir.dt.bfloat16
F8 = mybir.dt.float8e4
AF = mybir.ActivationFunctionType
ALU = mybir.AluOpType

P = 128                          # SBUF partitions
N_TOTAL = 16777216
N_CORES = 8
R = N_TOTAL // N_CORES           # pairs per core = 2097152
W = R // P                       # pairs per partition = 16384

TILE_C = 4096                    # pair-columns per tile
NT = W // TILE_C                 # 4 tiles
MM_C = 512                       # matmul chunk = one PSUM bank row

IO_BUFS = 3
MID_BUFS = 2


def _split_excess_waits(nc, max_waits=1):
    """This walrus build's CoreV3 codegen caps sem-wait commands per
    instruction; split excess waits onto preceding same-engine no-ops.
    Engines run their stream in order and the waits are monotonic
    sem-ge conditions, so sequential chunked waits are equivalent."""
    counter = [0]

    def fresh_name(base):
        counter[0] += 1
        return f"{base}-wsplit{counter[0]}"

    for fn in nc.m.functions:
        for bb in fn.blocks:
            out = []
            changed = False
            for inst in bb.instructions:
                si = inst.sync_info
                waits = list(si.on_wait) if si is not None else []
                if len(waits) > max_waits:
                    changed = True
                    head, tail = waits[:-max_waits], waits[-max_waits:]
                    for i in range(0, len(head), max_waits):
                        out.append(mybir.InstNoOp(
                            name=fresh_name(inst.name),
                            sync_info=mybir.SyncInfo(
                                on_wait=head[i:i + max_waits], on_update=[]),
                            bass_nofuse=True,
                            engine=inst.engine,
                        ))
                    inst.sync_info = mybir.SyncInfo(
                        on_wait=tail, on_update=list(si.on_update))
                out.append(inst)
            if changed:
                bb.instructions = out


def _build(io_bufs=IO_BUFS, mid_bufs=MID_BUFS):
    C = TILE_C
    nt = NT
    nc = bass.Bass(trn_type="TRN2", target_bir_lowering=False, debug=False)
    p0 = nc.dram_tensor("p0", [P, W], BF16, kind="ExternalInput").ap()
    p1 = nc.dram_tensor("p1", [P, W], BF16, kind="ExternalInput").ap()
    sg = nc.dram_tensor("sg", [P, W], F8, kind="ExternalInput").ap()
    # [ SA | SB ] per tile from the ScalarE accumulators
    out_acc = nc.dram_tensor("out_acc", [P, 2 * nt], F32,
                             kind="ExternalOutput").ap()
    # sum(u) + 2*sum(b) PE psum row (partition 0)
    out_red = nc.dram_tensor("out_red", [1, MM_C], F32,
                             kind="ExternalOutput").ap()

    ones = nc.const_aps.tensor(1.0, (P, 1), BF16)

    with tile.TileContext(nc) as tc:
        with tc.tile_pool(name="io", bufs=io_bufs) as io_pool, \
             tc.tile_pool(name="mid", bufs=mid_bufs) as mid_pool, \
             tc.tile_pool(name="accs", bufs=1) as acc_pool, \
             tc.psum_pool(name="red", bufs=1) as red_pool:
            twos_t = acc_pool.tile([P, 1], BF16)
            nc.vector.memset(twos_t[:], 2.0)
            acc = acc_pool.tile([P, 2 * nt], F32)
            red = red_pool.tile([1, MM_C], F32)
            red_sb = acc_pool.tile([1, MM_C], F32)
            for i in range(nt):
                cs = slice(i * C, (i + 1) * C)
                P0b = io_pool.tile([P, C], BF16, tag="P0b")
                P1b = io_pool.tile([P, C], BF16, tag="P1b")
                Sb = io_pool.tile([P, C], BF16, tag="Sb")
                nc.sync.dma_start(P0b[:], p0[:, cs])
                nc.sync.dma_start(P1b[:], p1[:, cs])
                # SWDGE cast-DMA: fp8 in HBM -> bf16 in SBUF
                nc.gpsimd.dma_start(Sb[:], sg[:, cs])

                scrA = mid_pool.tile([P, C], BF16, tag="scrA")
                eT = mid_pool.tile([P, C], BF16, tag="eT")
                uT = mid_pool.tile([P, C], BF16, tag="uT")
                bT = mid_pool.tile([P, C], BF16, tag="bT")

                # SA += sum (1 - p0)^2           [ScalarE, native accum]
                nc.scalar.activation(scrA[:], P0b[:], AF.Square,
                                     bias=1.0, scale=-1.0,
                                     accum_out=acc[:, i:i + 1])
                # SB += sum p1^2                 [ScalarE, native accum]
                nc.scalar.activation(scrA[:], P1b[:], AF.Square,
                                     accum_out=acc[:, nt + i:nt + i + 1])
                # e = p0 - p1                    [VectorE TT 2x]
                nc.vector.tensor_tensor(eT[:], P0b[:], P1b[:], ALU.subtract)
                # u = sigma * e                  [VectorE TT 2x]
                nc.vector.tensor_tensor(uT[:], Sb[:], eT[:], ALU.mult)
                # b = [u > e] (alt >= on odd tiles for unbiased ties)
                cmp = ALU.is_gt if i % 2 == 0 else ALU.is_ge
                nc.vector.tensor_tensor(bT[:], uT[:], eT[:], cmp)

                # RED += ones^T @ u-chunks + twos^T @ b-chunks, one PSUM
                # row; back-to-back matmuls keep the PE clock ramped.
                for w, src in ((ones, uT), (twos_t[:], bT)):
                    for mm in range(C // MM_C):
                        nc.tensor.matmul(
                            red[:, :], w,
                            src[:, mm * MM_C:(mm + 1) * MM_C],
                            start=(i == 0 and src is uT and mm == 0),
                            stop=(i == nt - 1 and src is bT
                                  and mm == C // MM_C - 1),
                        )

            # PSUM has no DMA route: bounce through SBUF.
            nc.vector.tensor_copy(red_sb[:], red[:])
            nc.sync.dma_start(out_acc[:], acc[:])
            nc.sync.dma_start(out_red[:], red_sb[:])

    _split_excess_waits(nc, max_waits=1)
    return nc, nt


_CACHE = {}


def _get_program():
    if "prog" not in _CACHE:
        _CACHE["prog"] = _build()
    return _CACHE["prog"]


def kernel(pred, target):
    pred = np.asarray(pred)
    target = np.asarray(target)
    assert pred.shape == (N_TOTAL, 2) and pred.dtype == np.float32
    bf16 = ml_dtypes.bfloat16
    f8 = ml_dtypes.float8_e4m3
    p0_h = np.ascontiguousarray(pred[:, 0]).astype(bf16)
    p1_h = np.ascontiguousarray(pred[:, 1]).astype(bf16)
    s_h = (2 * target.astype(np.int32)).astype(np.float32).astype(f8)

    nc, nt = _get_program()
    in_maps = []
    for c in range(N_CORES):
        sl = slice(c * R, (c + 1) * R)
        in_maps.append({
            "p0": p0_h[sl].reshape(P, W),
            "p1": p1_h[sl].reshape(P, W),
            "sg": s_h[sl].reshape(P, W),
        })

    res = run_bass_kernel_spmd(nc, in_maps, list(range(N_CORES)))

    total = 0.0
    for r in res.results:
        acc = np.asarray(r["out_acc"]).astype(np.float64)
        red = np.asarray(r["out_red"]).astype(np.float64)
        total += acc.sum() + red.sum()
    return np.float32(total / N_TOTAL)
